# revision 34
# baseline (speedup 1.0000x reference)
"""EvidNets Dempster-Shafer evidential classifier kernel for 8x TRN2 cores.

Reformulation: the sequential prototype scan has the closed form
    mk_n(b)   = prod_k (1 - s_kb)
    mk_c(b)   = prod_k (1 - s_kb * V_kc) - mk_n(b),   V = 1 - U
so with  L_c = ln prod_k (1 - s*V_c) = -sum_j (1/j) * sum_k V_kc^j s_kb^j
(series in s; s_max ~ 0.11 so J=5 converges below f32 scan noise),
everything becomes matmuls over the prototype dim plus ACT exp evals:
    T    = W@x.T - 0.5*||x||^2            (PE, bf16, exact x2 via hi/lo rows)
    s^j  = exp(j*g2*T + j*bias_k)         (ACT for j=1,2; DVE products after)
    L    = sum_j (-V^j/j).T @ s^j         (PE, accumulated in PSUM)
    out  = normalize(exp(L))              (ACT + tiny PE transposes + DVE)
"""

import numpy as np
import ml_dtypes

BF16 = ml_dtypes.bfloat16

B, D, P, C = 16384, 256, 512, 20
NCORES = 8
BPC = B // NCORES  # 2048
J = 3              # series order (J=3 truncation is below bf16 noise)
PT = P // 128      # 4 prototype tiles
NSPLIT = 512       # matmul free-dim split (one PSUM bank)
NT = BPC // NSPLIT # 4
HW = BPC // 2      # half-width for double-buffered T PSUM tiles
NH = HW // NSPLIT  # 2 n-splits per half
BT = BPC // 128    # 16

WCOLS = 2 * P                    # WT chunks in wpack
VOFF = WCOLS                     # vco offset in wpack
SBW = PT * 2 * J                 # scl/bia cols in sb
IDOFF = SBW                      # identity offset in sb

_cache = {}


def _build_bass():
    import concourse.bacc as bacc
    import concourse.mybir as mybir
    from concourse.tile import TileContext
    from contextlib import ExitStack

    dt = mybir.dt
    fp32 = dt.float32
    bf16 = dt.bfloat16

    nc = bacc.Bacc()

    # ---- DRAM parameters ----
    xT_d = nc.declare_dram_parameter("xT", [D, BPC], bf16, isOutput=False)
    xx_d = nc.declare_dram_parameter("xx", [2, P + BPC], bf16, isOutput=False)
    wpw_d = nc.declare_dram_parameter("wpw", [128, WCOLS], bf16, isOutput=False)
    wpv_d = nc.declare_dram_parameter(
        "wpv", [128, J * PT * (C + 1)], bf16, isOutput=False)
    sb_d = nc.declare_dram_parameter("sb", [128, SBW + C + 1], fp32,
                                     isOutput=False)
    out_d = nc.declare_dram_parameter("out", [BPC, C + 1], fp32, isOutput=True)

    with TileContext(nc) as tc:
        with (
            tc.tile_pool(name="consts", bufs=1) as consts,
            tc.tile_pool(name="sjpool", bufs=2) as sjpool,
            tc.tile_pool(name="epool", bufs=1) as epool,
            tc.tile_pool(name="fin", bufs=1) as fin,
            tc.tile_pool(name="psL", bufs=1, space="PSUM") as psL,
            ExitStack() as psT_stack,
        ):
            psT = psT_stack.enter_context(
                tc.tile_pool(name="psT", bufs=2, space="PSUM"))

            # ---- input/const DMAs: dispatch across four idle sequencers so
            # the ~650ns per-dma_start dispatch costs overlap ----
            xx = consts.tile([2, P + BPC], bf16, tag="xx")
            nc.sync.dma_start(out=xx, in_=xx_d[:, :])
            x2w = xx[:, 0:P]
            x2r = xx[:, P:P + BPC]
            xTs = [consts.tile([128, BPC], bf16, tag=f"xT{k}", name=f"xT{k}")
                   for k in range(2)]
            sbt = consts.tile([128, SBW + C + 1], fp32, tag="sbt")
            wpw = consts.tile([128, WCOLS], bf16, tag="wpw")
            wpv = consts.tile([128, J * PT * (C + 1)], bf16, tag="wpv")
            nc.sync.dma_start(out=wpw, in_=wpw_d[:, :])
            for k in range(2):
                nc.sync.dma_start(
                    out=xTs[k][:, 0:HW], in_=xT_d[k * 128:(k + 1) * 128, 0:HW])
            nc.sync.dma_start(out=sbt, in_=sb_d[:, :])
            nc.sync.dma_start(out=wpv, in_=wpv_d[:, :])
            for k in range(2):
                nc.sync.dma_start(
                    out=xTs[k][:, HW:BPC],
                    in_=xT_d[k * 128:(k + 1) * 128, HW:BPC])

            def wt(k):          # [128, P] WT chunk
                return wpw[:, k * P:(k + 1) * P]

            def vco(j, pt):     # [128, C+1] series lhsT for (j, pt)
                off = ((j - 1) * PT + pt) * (C + 1)
                return wpv[:, off:off + C + 1]

            def scl(pt, j):     # [128, 1]
                off = (pt * 2) * J + (j - 1)
                return sbt[:, off:off + 1]

            def bia(pt, j):     # [128, 1]
                off = (pt * 2 + 1) * J + (j - 1)
                return sbt[:, off:off + 1]

            idn = sbt[0:C + 1, IDOFF:IDOFF + C + 1]

            # L accumulator in PSUM: [21, BPC]
            Lps = psL.tile([C + 1, BPC], fp32)

            first_mm = [True] * NT  # per-n accumulation group start flag

            for pt in range(PT):
                ms = slice(pt * 128, (pt + 1) * 128)
                # ---- T = WT.T @ xT - 0.5*x2, in two half-width PSUM tiles
                # (bufs=2 so pt+1's matmuls overlap pt's ACT consumption);
                # s = exp(g2*T + bias) straight out of PSUM ----
                s1 = sjpool.tile([128, BPC], bf16, tag="sj1")
                for h in range(2):
                    hs = slice(h * HW, (h + 1) * HW)
                    Tps = psT.tile([128, HW], fp32, tag="T")
                    # x2 chunk first: it needs only the tiny DMAs, so PE can
                    # start (and HAM-warm) while the big xT halves stream in
                    for n in range(NH):
                        ns = slice(n * NSPLIT, (n + 1) * NSPLIT)
                        nc.tensor.matmul(
                            Tps[:, ns], x2w[:, ms],
                            x2r[:, h * HW + n * NSPLIT:
                                h * HW + (n + 1) * NSPLIT],
                            start=True, stop=False,
                        )
                    for k in range(2):
                        for n in range(NH):
                            ns = slice(n * NSPLIT, (n + 1) * NSPLIT)
                            nc.tensor.matmul(
                                Tps[:, ns], wt(k)[:, ms],
                                xTs[k][:, h * HW + n * NSPLIT:
                                       h * HW + (n + 1) * NSPLIT],
                                start=False, stop=(k == 1),
                            )
                    nc.scalar.activation(
                        out=s1[:, hs], in_=Tps,
                        func=mybir.ActivationFunctionType.Exp,
                        scale=scl(pt, 1), bias=bia(pt, 1),
                    )
                sj = {1: s1}
                for j in range(2, J + 1):
                    t = sjpool.tile([128, BPC], bf16, tag=f"sj{j}")
                    a, b = (j - 1, 1) if j != 4 else (2, 2)
                    nc.vector.tensor_mul(t, sj[a], sj[b])
                    sj[j] = t
                # ---- series matmuls: L += (-V^j/j).T @ s^j ----
                for j in range(1, J + 1):
                    for n in range(NT):
                        ns = slice(n * NSPLIT, (n + 1) * NSPLIT)
                        is_last = (pt == PT - 1) and (j == J)
                        nc.tensor.matmul(
                            Lps[:, ns], vco(j, pt), sj[j][:, ns],
                            start=first_mm[n], stop=is_last,
                        )
                        first_mm[n] = False

            # ---- E = exp(L) ----
            E = epool.tile([C + 1, BPC], fp32, tag="E")
            for n in range(2):
                ns = slice(n * HW, (n + 1) * HW)
                nc.scalar.activation(
                    out=E[:, ns], in_=Lps[:, ns],
                    func=mybir.ActivationFunctionType.Exp,
                )

            # release T's PSUM banks, then take one bank for the transposes
            psT_stack.close()
            psO_cm = tc.tile_pool(name="psO", bufs=1, space="PSUM")
            psO = psO_cm.__enter__()

            # ---- transpose to [128, 16*21] in one PSUM bank ----
            mkT = psO.tile([128, BT, C + 1], fp32)
            for bt in range(BT):
                nc.tensor.transpose(
                    mkT[:, bt, :], E[:, bt * 128:(bt + 1) * 128], idn,
                )

            # ---- normalize: out_c = (E_c - E_n)/K, out_n = E_n/K ----
            # K = sum_{c<20} E_c - 19*E_n
            e20 = fin.tile([128, BT], fp32, tag="e20")
            nc.vector.tensor_copy(e20, mkT[:, :, C])
            ssum = fin.tile([128, BT], fp32, tag="ssum")
            nc.vector.reduce_sum(ssum, mkT[:, :, 0:C], axis=mybir.AxisListType.X)
            kk = fin.tile([128, BT], fp32, tag="kk")
            nc.vector.tensor_scalar(
                out=kk, in0=e20, scalar1=float(-(C - 1)), scalar2=None,
                op0=mybir.AluOpType.mult,
            )
            nc.vector.tensor_add(kk, kk, ssum)
            rk = fin.tile([128, BT], fp32, tag="rk")
            nc.vector.reciprocal(rk, kk)
            numer = fin.tile([128, BT, C], fp32, tag="numer")
            nc.vector.tensor_sub(
                numer, mkT[:, :, 0:C], e20.to_broadcast((128, BT, C))
            )
            outt = fin.tile([128, BT, C + 1], fp32, tag="outt")
            nc.vector.tensor_mul(
                outt[:, :, 0:C], numer, rk.to_broadcast((128, BT, C))
            )
            nc.vector.tensor_mul(outt[:, :, C], e20, rk)
            # ---- store: out[(bt,p), c] ----
            nc.sync.dma_start(
                out=out_d.rearrange("(t p) c -> p t c", p=128), in_=outt
            )
            psO_cm.__exit__(None, None, None)

    nc.finalize()
    return nc


def _host_prep(inputs, W, BETA, alpha, gamma):
    """Host-side packing: shard x over cores, precompute small tensors."""
    x = np.asarray(inputs, dtype=np.float32)
    W = np.asarray(W, dtype=np.float32)
    BETA = np.asarray(BETA, dtype=np.float32)
    alpha = np.asarray(alpha, dtype=np.float32).reshape(P, 1)
    gamma = np.asarray(gamma, dtype=np.float32).reshape(P, 1)

    B2 = BETA.astype(np.float64) ** 2
    U = B2 / B2.sum(1, keepdims=True)
    Vaug = np.concatenate([1.0 - U, np.ones((P, 1))], 1)    # [P, C+1]
    alphap = 0.99 / (1.0 + np.exp(-alpha.astype(np.float64)))
    g2 = gamma.astype(np.float64) ** 2                      # [P,1]
    w2 = (W.astype(np.float64) ** 2).sum(1, keepdims=True)  # [P,1]

    # per-j ACT affine: s^j = exp(j*g2*T + j*(ln alphap - g2*(0.5*w2 + 128)))
    js = np.arange(1, J + 1, dtype=np.float64)[None, :]
    scl = (js * g2).astype(np.float32)                      # [P, J]
    bia = (js * (np.log(alphap) - g2 * (0.5 * w2 + 128.0))).astype(np.float32)

    # wpw: WT chunks; wpv: vco series coefficients (sign folded in)
    wpw = np.zeros((128, WCOLS), dtype=BF16)
    WTb = np.ascontiguousarray(W.T).astype(BF16)            # [D, P]
    for k in range(2):
        wpw[:, k * P:(k + 1) * P] = WTb[k * 128:(k + 1) * 128, :]
    wpv = np.zeros((128, J * PT * (C + 1)), dtype=BF16)
    for j in range(1, J + 1):
        co = (-(Vaug ** j) / j).astype(BF16)                # [P, C+1]
        for pt in range(PT):
            off = ((j - 1) * PT + pt) * (C + 1)
            wpv[:, off:off + C + 1] = co[pt * 128:(pt + 1) * 128, :]

    # sb: [128, SBW + C + 1] fp32 = interleaved scl/bia per pt, then eye(21)
    sb = np.zeros((128, SBW + C + 1), dtype=np.float32)
    for pt in range(PT):
        sb[:, (pt * 2) * J:(pt * 2) * J + J] = scl[pt * 128:(pt + 1) * 128, :]
        sb[:, (pt * 2 + 1) * J:(pt * 2 + 1) * J + J] = \
            bia[pt * 128:(pt + 1) * 128, :]
    sb[0:C + 1, IDOFF:IDOFF + C + 1] = np.eye(C + 1, dtype=np.float32)

    xb = x.astype(BF16)
    x2 = (x.astype(np.float64) ** 2).sum(1)                 # [B]
    x2c = x2 - 256.0
    x2_hi = x2c.astype(BF16)
    x2_lo = (x2c - x2_hi.astype(np.float64)).astype(BF16)

    shared = dict(wpw=wpw, wpv=wpv, sb=sb)
    in_maps = []
    for i in range(NCORES):
        bs = slice(i * BPC, (i + 1) * BPC)
        xTi = np.ascontiguousarray(xb[bs].T)                # [D, BPC] bf16
        xxi = np.full((2, P + BPC), -0.5, dtype=BF16)
        xxi[0, P:] = x2_hi[bs]
        xxi[1, P:] = x2_lo[bs]
        in_maps.append(dict(xT=xTi, xx=xxi, **shared))
    return in_maps


def kernel(inputs, W, BETA, alpha, gamma, n_class=None, prototype_dim=None,
           **_ignored):
    from concourse.bass_utils import run_bass_kernel_spmd

    if "nc" not in _cache:
        _cache["nc"] = _build_bass()
    nc = _cache["nc"]

    in_maps = _host_prep(inputs, W, BETA, alpha, gamma)
    res = run_bass_kernel_spmd(nc, in_maps, core_ids=list(range(NCORES)))
    outs = [np.asarray(res.results[i]["out"]) for i in range(NCORES)]
    return np.concatenate(outs, axis=0).astype(np.float32)


# revision 38
# speedup vs baseline: 2161.8182x; 2161.8182x over previous
"""EvidNets Dempster-Shafer evidential classifier kernel for 8x TRN2 cores.

Reformulation: the sequential prototype scan has the closed form
    mk_n(b)   = prod_k (1 - s_kb)
    mk_c(b)   = prod_k (1 - s_kb * V_kc) - mk_n(b),   V = 1 - U
so with  L_c = ln prod_k (1 - s*V_c) = -sum_j (1/j) * sum_k V_kc^j s_kb^j
(series in s; s_max ~ 0.11 so J=5 converges below f32 scan noise),
everything becomes matmuls over the prototype dim plus ACT exp evals:
    T    = W@x.T - 0.5*||x||^2            (PE, bf16, exact x2 via hi/lo rows)
    s^j  = exp(j*g2*T + j*bias_k)         (ACT for j=1,2; DVE products after)
    L    = sum_j (-V^j/j).T @ s^j         (PE, accumulated in PSUM)
    out  = normalize(exp(L))              (ACT + tiny PE transposes + DVE)
"""

import numpy as np
import ml_dtypes

BF16 = ml_dtypes.bfloat16

B, D, P, C = 16384, 256, 512, 20
NCORES = 8
BPC = B // NCORES  # 2048
J = 3              # series order (J=3 truncation is below bf16 noise)
PT = P // 128      # 4 prototype tiles
NSPLIT = 512       # matmul free-dim split (one PSUM bank)
NT = BPC // NSPLIT # 4
HW = BPC // 2      # half-width for double-buffered T PSUM tiles
NH = HW // NSPLIT  # 2 n-splits per half
BT = BPC // 128    # 16

WCOLS = 2 * P                    # WT chunks in wpack
VOFF = WCOLS                     # vco offset in wpack
SBW = PT * 2 * J                 # scl/bia cols in sb
IDOFF = SBW                      # identity offset in sb

_cache = {}


def _build_bass():
    import concourse.bacc as bacc
    import concourse.mybir as mybir
    from concourse.tile import TileContext
    from contextlib import ExitStack

    dt = mybir.dt
    fp32 = dt.float32
    bf16 = dt.bfloat16

    nc = bacc.Bacc()

    # ---- DRAM parameters ----
    xT_d = nc.declare_dram_parameter("xT", [D, BPC], bf16, isOutput=False)
    xx_d = nc.declare_dram_parameter("xx", [2, P + BPC], bf16, isOutput=False)
    wpw_d = nc.declare_dram_parameter("wpw", [128, WCOLS], bf16, isOutput=False)
    wpv_d = nc.declare_dram_parameter(
        "wpv", [128, J * PT * (C + 1)], bf16, isOutput=False)
    sb_d = nc.declare_dram_parameter("sb", [128, SBW + C + 1], fp32,
                                     isOutput=False)
    out_d = nc.declare_dram_parameter("out", [BPC, C + 1], fp32, isOutput=True)

    with TileContext(nc) as tc:
        with (
            tc.tile_pool(name="consts", bufs=1) as consts,
            tc.tile_pool(name="sjpool", bufs=3) as sjpool,
            tc.tile_pool(name="epool", bufs=1) as epool,
            tc.tile_pool(name="fin", bufs=1) as fin,
            tc.tile_pool(name="psL", bufs=1, space="PSUM") as psL,
            ExitStack() as psT_stack,
        ):
            psT = psT_stack.enter_context(
                tc.tile_pool(name="psT", bufs=2, space="PSUM"))

            # ---- input/const DMAs: dispatch across four idle sequencers so
            # the ~650ns per-dma_start dispatch costs overlap ----
            xx = consts.tile([2, P + BPC], bf16, tag="xx")
            nc.sync.dma_start(out=xx, in_=xx_d[:, :])
            x2w = xx[:, 0:P]
            x2r = xx[:, P:P + BPC]
            xTs = [consts.tile([128, BPC], bf16, tag=f"xT{k}", name=f"xT{k}")
                   for k in range(2)]
            sbt = consts.tile([128, SBW + C + 1], fp32, tag="sbt")
            wpw = consts.tile([128, WCOLS], bf16, tag="wpw")
            wpv = consts.tile([128, J * PT * (C + 1)], bf16, tag="wpv")
            nc.sync.dma_start(out=wpw, in_=wpw_d[:, :])
            nc.sync.dma_start(out=sbt, in_=sb_d[:, :])
            for h in range(2):
                for k in range(2):
                    hs = slice(h * HW, (h + 1) * HW)
                    nc.sync.dma_start(
                        out=xTs[k][:, hs], in_=xT_d[k * 128:(k + 1) * 128, hs])
            nc.sync.dma_start(out=wpv, in_=wpv_d[:, :])

            def wt(k):          # [128, P] WT chunk
                return wpw[:, k * P:(k + 1) * P]

            def vco(j, pt):     # [128, C+1] series lhsT for (j, pt)
                off = ((j - 1) * PT + pt) * (C + 1)
                return wpv[:, off:off + C + 1]

            def scl(pt, j):     # [128, 1]
                off = (pt * 2) * J + (j - 1)
                return sbt[:, off:off + 1]

            def bia(pt, j):     # [128, 1]
                off = (pt * 2 + 1) * J + (j - 1)
                return sbt[:, off:off + 1]

            idn = sbt[0:C + 1, IDOFF:IDOFF + C + 1]

            # L accumulator in PSUM: [21, BPC]
            Lps = psL.tile([C + 1, BPC], fp32)

            first_mm = [True] * NT  # per-n accumulation group start flag

            for pt in range(PT):
                ms = slice(pt * 128, (pt + 1) * 128)
                # ---- T = WT.T @ xT - 0.5*x2, in two half-width PSUM tiles
                # (bufs=2 so pt+1's matmuls overlap pt's ACT consumption);
                # s = exp(g2*T + bias) straight out of PSUM ----
                s1 = sjpool.tile([128, BPC], bf16, tag="sj1")
                for h in range(2):
                    hs = slice(h * HW, (h + 1) * HW)
                    Tps = psT.tile([128, HW], fp32, tag="T")
                    # x2 chunk first: it needs only the tiny DMAs, so PE can
                    # start (and HAM-warm) while the big xT halves stream in
                    for n in range(NH):
                        ns = slice(n * NSPLIT, (n + 1) * NSPLIT)
                        nc.tensor.matmul(
                            Tps[:, ns], x2w[:, ms],
                            x2r[:, h * HW + n * NSPLIT:
                                h * HW + (n + 1) * NSPLIT],
                            start=True, stop=False,
                        )
                    for k in range(2):
                        for n in range(NH):
                            ns = slice(n * NSPLIT, (n + 1) * NSPLIT)
                            nc.tensor.matmul(
                                Tps[:, ns], wt(k)[:, ms],
                                xTs[k][:, h * HW + n * NSPLIT:
                                       h * HW + (n + 1) * NSPLIT],
                                start=False, stop=(k == 1),
                            )
                    nc.scalar.activation(
                        out=s1[:, hs], in_=Tps,
                        func=mybir.ActivationFunctionType.Exp,
                        scale=scl(pt, 1), bias=bia(pt, 1),
                    )
                sj = {1: s1}
                for j in range(2, J + 1):
                    t = sjpool.tile([128, BPC], bf16, tag=f"sj{j}")
                    a, b = (j - 1, 1) if j != 4 else (2, 2)
                    for h in range(2):
                        hs = slice(h * HW, (h + 1) * HW)
                        nc.vector.tensor_mul(t[:, hs], sj[a][:, hs],
                                             sj[b][:, hs])
                    sj[j] = t
                # ---- series matmuls: L += (-V^j/j).T @ s^j ----
                for j in range(1, J + 1):
                    for n in range(NT):
                        ns = slice(n * NSPLIT, (n + 1) * NSPLIT)
                        is_last = (pt == PT - 1) and (j == J)
                        nc.tensor.matmul(
                            Lps[:, ns], vco(j, pt), sj[j][:, ns],
                            start=first_mm[n], stop=is_last,
                        )
                        first_mm[n] = False

            # ---- E = exp(L) ----
            E = epool.tile([C + 1, BPC], fp32, tag="E")
            for n in range(2):
                ns = slice(n * HW, (n + 1) * HW)
                nc.scalar.activation(
                    out=E[:, ns], in_=Lps[:, ns],
                    func=mybir.ActivationFunctionType.Exp,
                )

            # release T's PSUM banks, then take one bank for the transposes
            psT_stack.close()
            psO_cm = tc.tile_pool(name="psO", bufs=1, space="PSUM")
            psO = psO_cm.__enter__()

            # ---- transpose to [128, 16*21] in one PSUM bank ----
            mkT = psO.tile([128, BT, C + 1], fp32)
            for bt in range(BT):
                nc.tensor.transpose(
                    mkT[:, bt, :], E[:, bt * 128:(bt + 1) * 128], idn,
                )

            # ---- normalize: out_c = (E_c - E_n)/K, out_n = E_n/K ----
            # K = sum_{c<20} E_c - 19*E_n
            e20 = fin.tile([128, BT], fp32, tag="e20")
            nc.vector.tensor_copy(e20, mkT[:, :, C])
            ssum = fin.tile([128, BT], fp32, tag="ssum")
            nc.vector.reduce_sum(ssum, mkT[:, :, 0:C], axis=mybir.AxisListType.X)
            kk = fin.tile([128, BT], fp32, tag="kk")
            nc.vector.tensor_scalar(
                out=kk, in0=e20, scalar1=float(-(C - 1)), scalar2=None,
                op0=mybir.AluOpType.mult,
            )
            nc.vector.tensor_add(kk, kk, ssum)
            rk = fin.tile([128, BT], fp32, tag="rk")
            nc.vector.reciprocal(rk, kk)
            numer = fin.tile([128, BT, C], fp32, tag="numer")
            nc.vector.tensor_sub(
                numer, mkT[:, :, 0:C], e20.to_broadcast((128, BT, C))
            )
            outt = fin.tile([128, BT, C + 1], fp32, tag="outt")
            nc.vector.tensor_mul(
                outt[:, :, 0:C], numer, rk.to_broadcast((128, BT, C))
            )
            nc.vector.tensor_mul(outt[:, :, C], e20, rk)
            # ---- store: out[(bt,p), c] ----
            nc.sync.dma_start(
                out=out_d.rearrange("(t p) c -> p t c", p=128), in_=outt
            )
            psO_cm.__exit__(None, None, None)

    nc.finalize()
    return nc


def _host_prep(inputs, W, BETA, alpha, gamma):
    """Host-side packing: shard x over cores, precompute small tensors."""
    x = np.asarray(inputs, dtype=np.float32)
    W = np.asarray(W, dtype=np.float32)
    BETA = np.asarray(BETA, dtype=np.float32)
    alpha = np.asarray(alpha, dtype=np.float32).reshape(P, 1)
    gamma = np.asarray(gamma, dtype=np.float32).reshape(P, 1)

    B2 = BETA.astype(np.float64) ** 2
    U = B2 / B2.sum(1, keepdims=True)
    Vaug = np.concatenate([1.0 - U, np.ones((P, 1))], 1)    # [P, C+1]
    alphap = 0.99 / (1.0 + np.exp(-alpha.astype(np.float64)))
    g2 = gamma.astype(np.float64) ** 2                      # [P,1]
    w2 = (W.astype(np.float64) ** 2).sum(1, keepdims=True)  # [P,1]

    # per-j ACT affine: s^j = exp(j*g2*T + j*(ln alphap - g2*(0.5*w2 + 128)))
    js = np.arange(1, J + 1, dtype=np.float64)[None, :]
    scl = (js * g2).astype(np.float32)                      # [P, J]
    bia = (js * (np.log(alphap) - g2 * (0.5 * w2 + 128.0))).astype(np.float32)

    # wpw: WT chunks; wpv: vco series coefficients (sign folded in)
    wpw = np.zeros((128, WCOLS), dtype=BF16)
    WTb = np.ascontiguousarray(W.T).astype(BF16)            # [D, P]
    for k in range(2):
        wpw[:, k * P:(k + 1) * P] = WTb[k * 128:(k + 1) * 128, :]
    wpv = np.zeros((128, J * PT * (C + 1)), dtype=BF16)
    for j in range(1, J + 1):
        co = (-(Vaug ** j) / j).astype(BF16)                # [P, C+1]
        for pt in range(PT):
            off = ((j - 1) * PT + pt) * (C + 1)
            wpv[:, off:off + C + 1] = co[pt * 128:(pt + 1) * 128, :]

    # sb: [128, SBW + C + 1] fp32 = interleaved scl/bia per pt, then eye(21)
    sb = np.zeros((128, SBW + C + 1), dtype=np.float32)
    for pt in range(PT):
        sb[:, (pt * 2) * J:(pt * 2) * J + J] = scl[pt * 128:(pt + 1) * 128, :]
        sb[:, (pt * 2 + 1) * J:(pt * 2 + 1) * J + J] = \
            bia[pt * 128:(pt + 1) * 128, :]
    sb[0:C + 1, IDOFF:IDOFF + C + 1] = np.eye(C + 1, dtype=np.float32)

    xb = x.astype(BF16)
    x2 = (x.astype(np.float64) ** 2).sum(1)                 # [B]
    x2c = x2 - 256.0
    x2_hi = x2c.astype(BF16)
    x2_lo = (x2c - x2_hi.astype(np.float64)).astype(BF16)

    shared = dict(wpw=wpw, wpv=wpv, sb=sb)
    in_maps = []
    for i in range(NCORES):
        bs = slice(i * BPC, (i + 1) * BPC)
        xTi = np.ascontiguousarray(xb[bs].T)                # [D, BPC] bf16
        xxi = np.full((2, P + BPC), -0.5, dtype=BF16)
        xxi[0, P:] = x2_hi[bs]
        xxi[1, P:] = x2_lo[bs]
        in_maps.append(dict(xT=xTi, xx=xxi, **shared))
    return in_maps


def kernel(inputs, W, BETA, alpha, gamma, n_class=None, prototype_dim=None,
           **_ignored):
    from concourse.bass_utils import run_bass_kernel_spmd

    if "nc" not in _cache:
        _cache["nc"] = _build_bass()
    nc = _cache["nc"]

    in_maps = _host_prep(inputs, W, BETA, alpha, gamma)
    res = run_bass_kernel_spmd(nc, in_maps, core_ids=list(range(NCORES)))
    outs = [np.asarray(res.results[i]["out"]) for i in range(NCORES)]
    return np.concatenate(outs, axis=0).astype(np.float32)
